# revision 32
# baseline (speedup 1.0000x reference)
"""Trainium2 Bass kernel for nn_DAWNLayer (moe_routing).

Strategy: data-parallel over batch B=8 across the 8 NeuronCores; each core
runs the full layer for one [S=1024, D=768] slice. Weights are replicated.

Key algorithmic facts used (verified numerically against the reference):
  * attn.mean(-1) of a softmax over the same axis is the constant 1/S, so
    `context` is a per-neuron constant: sigmoid(ctx_pat.sum(-1)/S). It is
    folded into the neuron-embedding matrix (scales recipe_norm rows).
  * top-k(8) + softmax + gather reduces to a thresholded masked softmax over
    the 96 neuron scores followed by a dense [S,96] @ [96,32] matmul
    (vector.max returns the top-8 values per row in one DVE instruction).
  * softmax without max-subtraction is safe here (|scores/8| < ~6).

Numerics: bf16 matmuls (fp32 PSUM accumulation) everywhere except the routing
score path (semantic scores), which stays exact fp32 so top-8 selection
matches the reference.

Attention is computed K-major: scores^T = K_tile^T-major matmul so exp writes
the AV-ready [k, q] layout straight from PSUM (no PE transposes / SBUF
copies), V carries a ones-column per head so the AV matmul also emits the
softmax denominator, and normalization is deferred to one small multiply per
(head, q-half) after AV. Weight f32->bf16 casts are split across the scalar
and pool engines and issued early so they overlap compute instead of gating
the Q/K projections; FFN weights prefetch during attention.
"""

import os
import numpy as np

B, S, D = 8, 1024, 768
H, DH = 12, 64
DFF = 3072
NN, NB, RK, TOPK = 96, 32, 64, 8
P = 128
TILES = S // P          # 8 token tiles
DC = D // P             # 6 chunks of d-model
FC = DFF // P           # 24 chunks of d_ff
NCORES = 8

_CACHE = {}


def _build():
    import concourse.bass as bass
    import concourse.bacc as bacc
    import concourse.mybir as mybir
    from concourse.tile import TileContext
    from concourse.masks import make_identity
    from contextlib import ExitStack

    f32 = mybir.dt.float32
    bf16 = mybir.dt.bfloat16
    AF = mybir.ActivationFunctionType
    OP = mybir.AluOpType
    AX = mybir.AxisListType

    nc = bacc.Bacc("TRN2", target_bir_lowering=False, debug=False,
                   num_devices=NCORES)

    # ---- DRAM I/O ----------------------------------------------------------
    d_in = {}
    def din(name, shape):
        d_in[name] = nc.dram_tensor(name, list(shape), f32, kind="ExternalInput")
        return d_in[name]

    x_d = din("x", (S, D))
    qw_d = din("q_w", (D, D)); qb_d = din("q_b", (D,))
    kw_d = din("k_w", (D, D)); kb_d = din("k_b", (D,))
    aow_d = din("ao_w", (D, D)); aob_d = din("ao_b", (D,))
    recipe_d = din("recipe", (NN, NB))
    ctx_d = din("ctx_pat", (NN, H))
    bemb_d = din("basis_emb", (NB, D))
    bA_d = din("basis_A", (NB, D, RK))
    voutw_d = din("vout_w", (RK, D)); voutb_d = din("vout_b", (D,))
    upw_d = din("up_w", (D, DFF)); upb_d = din("up_b", (DFF,))
    downw_d = din("down_w", (DFF, D)); downb_d = din("down_b", (D,))
    n1g_d = din("n1_g", (D,)); n1b_d = din("n1_b", (D,))
    n2g_d = din("n2_g", (D,)); n2b_d = din("n2_b", (D,))
    y_d = nc.dram_tensor("y", [S, D], f32, kind="ExternalOutput")

    def bcast_row(src_handle):
        """DRAM [D] -> AP broadcasting along 128 partitions."""
        ap = src_handle.ap()
        return bass.AP(tensor=ap.tensor, offset=ap.offset,
                       ap=[[0, P]] + list(ap.ap))

    with TileContext(nc, pool_alloc_mode="queue") as tc, ExitStack() as top:
        psA = top.enter_context(tc.tile_pool(name="psA", bufs=2, space="PSUM"))
        psB = top.enter_context(tc.tile_pool(name="psB", bufs=3, space="PSUM"))
        psT = top.enter_context(tc.tile_pool(name="psT", bufs=1, space="PSUM"))
        singles = top.enter_context(tc.tile_pool(name="singles", side="left", bufs=1))
        work = top.enter_context(tc.tile_pool(name="work", side="left", bufs=2))
        stage = top.enter_context(tc.tile_pool(name="stage", side="left", bufs=2))
        xload = top.enter_context(tc.tile_pool(name="xload", side="left", bufs=1))

        def pA():   return psA.tile([P, 1024], f32, tag="psA", name="psA_t")
        def pB():   return psB.tile([P, 512], f32, tag="psB", name="psB_t")
        def pT(dt): return psT.tile([P, P], dt, tag="psT", name="psT_t")

        # ---- constants / small tables -------------------------------------
        id_f = singles.tile([P, P], f32)
        make_identity(nc, id_f)
        eps_t = singles.tile([P, 1], f32)
        nc.vector.memset(eps_t, 1e-5)

        ones_row = singles.tile([1, P], f32)
        nc.vector.memset(ones_row, 1.0)

        def bcast_via_pe(dram, name, dt=f32):
            row = stage.tile([1, D], f32, tag="stage8k", name=name + "_row")
            nc.sync.dma_start(row, dram.ap()[None, :])
            out = singles.tile([P, D], dt, name=name)
            ps = pA()
            nc.tensor.matmul(ps[:, 0:512], lhsT=ones_row, rhs=row[:, 0:512],
                             start=True, stop=True)
            nc.tensor.matmul(ps[:, 512:768], lhsT=ones_row, rhs=row[:, 512:768],
                             start=True, stop=True)
            nc.vector.tensor_copy(out=out, in_=ps[:, :D])
            return out

        g1 = bcast_via_pe(n1g_d, "g1")
        b1 = bcast_via_pe(n1b_d, "b1")
        g2 = bcast_via_pe(n2g_d, "g2", bf16)
        b2 = bcast_via_pe(n2b_d, "b2", bf16)
        aob = bcast_via_pe(aob_d, "aob", bf16)
        voutb = bcast_via_pe(voutb_d, "voutb", bf16)
        downb = bcast_via_pe(downb_d, "downb", bf16)
        qb = singles.tile([P, DC], f32)
        nc.sync.dma_start(qb, qb_d.ap().rearrange("(o p) -> p o", p=P))
        kb = singles.tile([P, DC], f32)
        nc.sync.dma_start(kb, kb_d.ap().rearrange("(o p) -> p o", p=P))
        upb = singles.tile([P, FC], f32)
        nc.sync.dma_start(upb, upb_d.ap().rearrange("(o p) -> p o", p=P))

        # routing tables
        recipe_sb = singles.tile([NN, NB], f32)
        nc.sync.dma_start(recipe_sb, recipe_d.ap())
        ctx_sb = singles.tile([NN, H], f32)
        nc.sync.dma_start(ctx_sb, ctx_d.ap())
        bemb_sb = stage.tile([NB, D], f32, tag="stage8k")
        nc.sync.dma_start(bemb_sb, bemb_d.ap())
        voutw_sb = stage.tile([RK, D], f32, tag="stage8k")
        nc.sync.dma_start(voutw_sb, voutw_d.ap())
        voutw_bf = singles.tile([RK, D], bf16)
        nc.gpsimd.tensor_copy(out=voutw_bf, in_=voutw_sb)

        # recipe softmax (rows)
        rmax = singles.tile([NN, 1], f32)
        nc.vector.tensor_reduce(out=rmax, in_=recipe_sb, axis=AX.X, op=OP.max)
        nrmax = singles.tile([NN, 1], f32)
        nc.vector.tensor_scalar_mul(nrmax, rmax, -1.0)
        recipe_norm = singles.tile([NN, NB], f32)
        rsum = singles.tile([NN, 1], f32)
        nc.scalar.activation(out=recipe_norm, in_=recipe_sb, func=AF.Exp,
                             bias=nrmax[:, 0:1], accum_out=rsum)
        rinv = singles.tile([NN, 1], f32)
        nc.vector.reciprocal(rinv, rsum)
        nc.vector.tensor_scalar_mul(recipe_norm, recipe_norm, rinv[:, 0:1])

        # constant context -> sigmoid, folded into scaled recipe rows
        csum = singles.tile([NN, 1], f32)
        nc.vector.tensor_reduce(out=csum, in_=ctx_sb, axis=AX.X, op=OP.add)
        sigc = singles.tile([NN, 1], f32)
        nc.scalar.activation(out=sigc, in_=csum, func=AF.Sigmoid, scale=1.0 / S)
        recipe_s = singles.tile([NN, NB], f32)
        nc.vector.tensor_scalar_mul(recipe_s, recipe_norm, sigc[:, 0:1])

        # recipe_s^T [NB, NN]
        ps = pT(f32)
        nc.tensor.transpose(ps[:NB, :NN], recipe_s, id_f[:NN, :NN])
        recipe_sT = singles.tile([NB, NN], f32)
        nc.vector.tensor_copy(out=recipe_sT, in_=ps[:NB, :NN])

        # neuron_embT_scaled [D(part), NN] = basis_emb^T @ recipe_s^T
        es_rt = ExitStack()
        p_rt = es_rt.enter_context(tc.tile_pool(name="p_rt", side="left", bufs=1))
        nembT = p_rt.tile([P, DC, NN], f32, tag="nembT")
        for c in range(DC):
            ps = pB()
            nc.tensor.matmul(ps[:P, :NN], lhsT=bemb_sb[:NB, c * P:(c + 1) * P],
                             rhs=recipe_sT[:NB, :NN], start=True, stop=True)
            nc.vector.tensor_copy(out=nembT[:, c, :], in_=ps[:P, :NN])

        # ---- Phase A: LN1 + transpose -------------------------------------
        es_lnf = ExitStack()
        p_lnf = es_lnf.enter_context(tc.tile_pool(name="p_lnf", side="left", bufs=1))
        es_lnb = ExitStack()
        p_lnb = es_lnb.enter_context(tc.tile_pool(name="p_lnb", side="right", bufs=1))
        normT = p_lnf.tile([P, DC, S], f32, tag="normT")
        normTb = p_lnb.tile([P, DC, S], bf16, tag="normTb")

        def layernorm_tile(xt, g, b, out_tile, aff=None):
            """token-major LN: xt [128, 768] -> out_tile [128, 768] f32."""
            if aff is None:
                aff = nc.vector
            stats = work.tile([P, 3, 6], f32, tag="ln_stats")
            xv = xt.rearrange("p (a q) -> p a q", a=3)
            for a in range(3):
                nc.vector.bn_stats(out=stats[:, a, :], in_=xv[:, a, :])
            mv = work.tile([P, 2], f32, tag="ln_mv")
            nc.vector.bn_aggr(out=mv, in_=stats)
            std = work.tile([P, 1], f32, tag="ln_std")
            nc.scalar.activation(out=std, in_=mv[:, 1:2], func=AF.Sqrt,
                                 bias=eps_t[:, 0:1])
            rstd = work.tile([P, 1], f32, tag="ln_rstd")
            nc.vector.reciprocal(rstd, std)
            nc.vector.tensor_scalar(out=out_tile, in0=xt, scalar1=mv[:, 0:1],
                                    scalar2=rstd[:, 0:1], op0=OP.subtract,
                                    op1=OP.mult)
            aff.tensor_tensor(out=out_tile, in0=out_tile, in1=g, op=OP.mult)
            aff.tensor_tensor(out=out_tile, in0=out_tile, in1=b, op=OP.add)

        x_re = x_d.ap().rearrange("(t p) d -> p t d", p=P)
        for t in range(TILES):
            xt = xload.tile([P, D], f32, tag="xt")
            nc.sync.dma_start(xt, x_re[:, t, :])
            nt = work.tile([P, D], f32, tag="nt")
            layernorm_tile(xt, g1, b1, nt)
            for g in range(2):
                ps = pB()
                for j in range(3):
                    c = g * 3 + j
                    nc.tensor.transpose(ps[:, j * P:(j + 1) * P],
                                        nt[:, c * P:(c + 1) * P], id_f)
                pv = ps[:, :3 * P].rearrange("p (j q) -> p j q", j=3)
                nc.scalar.activation(
                    out=normT[:, g * 3:(g + 1) * 3, t * P:(t + 1) * P], in_=pv,
                    func=AF.Identity)
        for c in range(DC):
            if c % 2 == 0:
                nc.scalar.activation(out=normTb[:, c, :], in_=normT[:, c, :],
                                     func=AF.Identity)
            else:
                nc.gpsimd.tensor_copy(out=normTb[:, c, :], in_=normT[:, c, :])

        # q/k weight load + bf16 cast, issued early so the casts fill the
        # scalar/pool idle windows during phases B/C and never gate Phase D.
        es_qkw = ExitStack()
        p_qkw = es_qkw.enter_context(tc.tile_pool(name="p_qkw", side="right", bufs=2))

        def load_w_bf(dram, shape_free, tag):
            """Load [D, N] fp32 weight as [128, DC, N] bf16 (split-engine cast)."""
            n = shape_free
            w_bf = p_qkw.tile([P, DC, n], bf16, tag=tag)
            re = dram.ap().rearrange("(o p) n -> p o n", p=P)
            for c in range(DC):
                st = stage.tile([P, 1, n], f32, tag="stage8k")
                nc.sync.dma_start(st[:, 0, :], re[:, c, :])
                if c % 2 == 0:
                    nc.scalar.activation(out=w_bf[:, c, :], in_=st[:, 0, :],
                                         func=AF.Identity)
                else:
                    nc.gpsimd.tensor_copy(out=w_bf[:, c, :], in_=st[:, 0, :])
            return w_bf

        qw_bf = load_w_bf(qw_d, D, "qkw")
        kw_bf = load_w_bf(kw_d, D, "qkw")

        # ---- Phase B: routing scores -> token_recipe ----------------------
        tr_sb = singles.tile([P, TILES, NB], f32)      # token-major token_recipe
        maskT = p_rt.tile([NN, S], f32, tag="maskT")   # mask_w^T
        for t in range(TILES):
            ps = pB()
            for c in range(DC):
                nc.tensor.matmul(ps[:P, :NN], lhsT=normT[:, c, t * P:(t + 1) * P],
                                 rhs=nembT[:, c, :], start=(c == 0),
                                 stop=(c == DC - 1))
            fin = work.tile([P, NN], f32, tag="fin")
            nc.scalar.activation(out=fin, in_=ps[:P, :NN], func=AF.Identity)
            mx = work.tile([P, 8], f32, tag="mx")
            nc.vector.max(out=mx, in_=fin)
            nmx = work.tile([P, 1], f32, tag="nmx")
            nc.vector.tensor_scalar_mul(nmx, mx[:, 0:1], -1.0)
            e = work.tile([P, NN], f32, tag="e")
            nc.scalar.activation(out=e, in_=fin, func=AF.Exp, bias=nmx[:, 0:1])
            msk = work.tile([P, NN], f32, tag="msk")
            nc.vector.tensor_scalar(out=msk, in0=fin, scalar1=mx[:, 7:8],
                                    scalar2=None, op0=OP.is_ge)
            nc.vector.tensor_tensor(out=e, in0=e, in1=msk, op=OP.mult)
            den = work.tile([P, 1], f32, tag="den")
            nc.vector.tensor_reduce(out=den, in_=e, axis=AX.X, op=OP.add)
            idn = work.tile([P, 1], f32, tag="idn")
            nc.vector.reciprocal(idn, den)
            nc.vector.tensor_scalar_mul(e, e, idn[:, 0:1])
            # transpose mask_w -> maskT
            ps2 = pT(f32)
            nc.tensor.transpose(ps2[:NN, :P], e[:, :NN], id_f)
            nc.vector.tensor_copy(out=maskT[:, t * P:(t + 1) * P], in_=ps2[:NN, :P])
        for t in range(TILES):
            ps = pB()
            nc.tensor.matmul(ps[:P, :NB], lhsT=maskT[:, t * P:(t + 1) * P],
                             rhs=recipe_norm, start=True, stop=True)
            nc.vector.tensor_copy(out=tr_sb[:, t, :], in_=ps[:P, :NB])
        es_lnf.close()
        es_rt.close()

        # ---- Phase C: proj / v_sem / V ------------------------------------
        es_bas = ExitStack()
        p_bas = es_bas.enter_context(tc.tile_pool(name="p_bas", side="left", bufs=1))
        basis_bf = p_bas.tile([P, DC, NB, RK], bf16, tag="basis")
        bA_re = bA_d.ap().rearrange("n (o p) r -> p o n r", p=P)
        for c in range(DC):
            for nh in range(2):
                st = stage.tile([P, NB // 2, RK], f32, tag="stage8k")
                nsl = slice(nh * (NB // 2), (nh + 1) * (NB // 2))
                nc.sync.dma_start(st, bA_re[:, c, nsl, :])
                if (c * 2 + nh) % 2 == 0:
                    nc.scalar.activation(out=basis_bf[:, c, nsl, :], in_=st,
                                         func=AF.Identity)
                else:
                    nc.gpsimd.tensor_copy(out=basis_bf[:, c, nsl, :], in_=st)

        vsemT_bf = singles.tile([RK, TILES, P], bf16)

        NGRP = 4           # groups of 8 basis entries -> 512-wide psum
        GN = NB // NGRP    # 8
        for t in range(TILES):
            vpart = work.tile([P, NGRP, RK], bf16, tag="vpart")
            for g in range(NGRP):
                ps = pB()
                for c in range(DC):
                    nc.tensor.matmul(
                        ps[:, :512],
                        lhsT=normTb[:, c, t * P:(t + 1) * P],
                        rhs=basis_bf[:, c, g * GN:(g + 1) * GN, :],
                        start=(c == 0), stop=(c == DC - 1))
                sc = work.tile([P, GN, RK], bf16, tag="sc")
                nc.vector.tensor_tensor(
                    out=sc, in0=ps.rearrange("p (n r) -> p n r", n=GN),
                    in1=tr_sb[:, t, g * GN:(g + 1) * GN, None].to_broadcast(
                        [P, GN, RK]),
                    op=OP.mult)
                nc.vector.tensor_tensor(out=sc[:, 0:4, :], in0=sc[:, 0:4, :],
                                        in1=sc[:, 4:8, :], op=OP.add)
                nc.vector.tensor_tensor(out=sc[:, 0:2, :], in0=sc[:, 0:2, :],
                                        in1=sc[:, 2:4, :], op=OP.add)
                nc.vector.tensor_tensor(out=vpart[:, g, :], in0=sc[:, 0, :],
                                        in1=sc[:, 1, :], op=OP.add)
            vsem = work.tile([P, RK], f32, tag="vsem")
            nc.vector.tensor_tensor(out=vsem, in0=vpart[:, 0, :],
                                    in1=vpart[:, 1, :], op=OP.add)
            nc.vector.tensor_tensor(out=vsem, in0=vsem, in1=vpart[:, 2, :],
                                    op=OP.add)
            nc.vector.tensor_tensor(out=vsem, in0=vsem, in1=vpart[:, 3, :],
                                    op=OP.add)
            ps2 = pT(f32)
            nc.tensor.transpose(ps2[:RK, :P], vsem, id_f)
            nc.vector.tensor_copy(out=vsemT_bf[:, t, :], in_=ps2[:RK, :P])
        es_bas.close()

        es_dn1 = ExitStack()
        p_dn1 = es_dn1.enter_context(tc.tile_pool(name="p_dn1", side="left", bufs=1))
        FC1 = 10
        downw_bf1 = p_dn1.tile([P, FC1, D], bf16, tag="downw1")
        downw_re = downw_d.ap().rearrange("(o p) n -> p o n", p=P)
        for c2 in range(FC1 // 2):
            st = stage.tile([P, 2, D], f32, tag="stage8k")
            nc.sync.dma_start(st, downw_re[:, c2 * 2:(c2 + 1) * 2, :])
            nc.gpsimd.tensor_copy(out=downw_bf1[:, c2 * 2:c2 * 2 + 2, :], in_=st)

        es_at = ExitStack()
        p_at = es_at.enter_context(tc.tile_pool(name="p_at", side="left", bufs=2))

        # V with a ones-column per head (col h*65+64) so the AV matmul also
        # produces the softmax denominator (row DH of each head's AV psum).
        es_v = ExitStack()
        p_v = es_v.enter_context(tc.tile_pool(name="p_v", side="left", bufs=1))
        EH = DH + 1
        V_ext = p_v.tile([P, TILES, H * EH], bf16, tag="V")
        ones_view = V_ext.rearrange("p t (h e) -> p t h e", e=EH)[:, :, :, DH:EH]
        nc.gpsimd.memset(ones_view, 1.0)
        for t in range(TILES):
            ps = pA()
            nc.tensor.matmul(ps[:, 0:512], lhsT=vsemT_bf[:, t, :],
                             rhs=voutw_bf[:, 0:512], start=True, stop=True)
            nc.tensor.matmul(ps[:, 512:768], lhsT=vsemT_bf[:, t, :],
                             rhs=voutw_bf[:, 512:768], start=True, stop=True)
            vv = V_ext[:, t, :].rearrange("p (h e) -> p h e", e=EH)
            nc.vector.tensor_tensor(
                out=vv[:, :, 0:DH],
                in0=ps[:, :768].rearrange("p (h e) -> p h e", e=DH),
                in1=voutb.rearrange("p (h e) -> p h e", e=DH),
                op=OP.add)

        # ---- Phase D: Q/K projections -------------------------------------
        es_qt = ExitStack()
        p_qt = es_qt.enter_context(tc.tile_pool(name="p_qt", side="left", bufs=2))

        QT = p_qt.tile([P, DC, S], bf16, tag="qkt")
        KT = p_qt.tile([P, DC, S], bf16, tag="qkt")
        for (w_bf, bias, out_t) in ((qw_bf, qb, QT), (kw_bf, kb, KT)):
            for m in range(DC):
                ps = pA()
                for half in range(2):
                    sl = slice(half * 512, (half + 1) * 512)
                    for c in range(DC):
                        nc.tensor.matmul(ps[:, sl],
                                         lhsT=w_bf[:, c, m * P:(m + 1) * P],
                                         rhs=normTb[:, c, sl],
                                         start=(c == 0), stop=(c == DC - 1))
                nc.scalar.activation(out=out_t[:, m, :], in_=ps,
                                     func=AF.Identity, bias=bias[:, m:m + 1])
        es_qkw.close()
        es_lnb.close()

        # prefetch FFN up weights during attention
        es_up = ExitStack()
        p_up = es_up.enter_context(tc.tile_pool(name="p_up", side="right", bufs=1))
        upw_bf = p_up.tile([P, DC, DFF], bf16, tag="upw")
        upw_re = upw_d.ap().rearrange("(o p) n -> p o n", p=P)
        for c in range(DC):
            for half in range(2):
                st = stage.tile([P, 1, DFF // 2], f32, tag="stage8k")
                sl = slice(half * (DFF // 2), (half + 1) * (DFF // 2))
                nc.sync.dma_start(st[:, 0, :], upw_re[:, c, sl])
                nc.gpsimd.tensor_copy(out=upw_bf[:, c, sl], in_=st[:, 0, :])

        # prefetch attn-output projection weights too
        es_aow = ExitStack()
        p_aow = es_aow.enter_context(tc.tile_pool(name="p_aow", side="right", bufs=1))
        aow_bf = p_aow.tile([P, DC, D], bf16, tag="aow")
        aow_re = aow_d.ap().rearrange("(o p) n -> p o n", p=P)
        for c in range(DC):
            st = stage.tile([P, 1, D], f32, tag="stage8k")
            nc.sync.dma_start(st[:, 0, :], aow_re[:, c, :])
            nc.gpsimd.tensor_copy(out=aow_bf[:, c, :], in_=st[:, 0, :])

        # ---- Phase E: attention (transposed scores, deferred normalize) ----
        # Scores are computed K-major ([k, q]) so exp writes the AV-ready
        # layout directly from PSUM (no PE transposes, no SBUF copies). The
        # softmax denominator comes from V's ones-column (row DH of the AV
        # psum); normalization is one PSUM*PSUM multiply per (h, q-half).
        es_aout = ExitStack()
        p_aout = es_aout.enter_context(tc.tile_pool(name="p_aout", side="right", bufs=1))
        aoutT = p_aout.tile([P, DC, S], bf16, tag="aoutT")
        for h in range(H):
            hp = (h % 2) * DH
            hc = h // 2
            attnT = p_at.tile([P, TILES, S], bf16, tag="attnT")
            for kt in range(TILES):
                ps = pA()
                for half in range(2):
                    sl = slice(half * 512, (half + 1) * 512)
                    nc.tensor.matmul(ps[:, sl],
                                     lhsT=KT[hp:hp + DH, hc, kt * P:(kt + 1) * P],
                                     rhs=QT[hp:hp + DH, hc, sl],
                                     start=True, stop=True)
                nc.scalar.activation(out=attnT[:, kt, :], in_=ps, func=AF.Exp,
                                     scale=0.125)
            for qc in range(2):
                qsl = slice(qc * 512, (qc + 1) * 512)
                ps = pB()
                for tb in range(TILES):
                    nc.tensor.matmul(ps[:DH + 1, :],
                                     lhsT=V_ext[:, tb, h * EH:(h + 1) * EH],
                                     rhs=attnT[:, tb, qsl],
                                     start=(tb == 0), stop=(tb == TILES - 1))
                av = work.tile([DH + 1, 512], f32, tag="dvb")
                nc.vector.tensor_copy(out=av, in_=ps[:DH + 1, :512])
                dvl = work.tile([1, 512], f32, tag="dv")
                nc.scalar.activation(out=dvl, in_=av[DH:DH + 1, :], func=AF.Ln)
                dv = work.tile([1, 512], f32, tag="dv")
                nc.scalar.activation(out=dv, in_=dvl, func=AF.Exp, scale=-1.0)
                dvb = work.tile([DH, 512], f32, tag="dvb")
                nc.gpsimd.partition_broadcast(dvb, dv)
                nc.vector.tensor_tensor(out=aoutT[hp:hp + DH, hc, qsl],
                                        in0=av[:DH, :], in1=dvb,
                                        op=OP.mult)
        es_qt.close()
        es_v.close()
        es_at.close()

        # ---- Phase F: attn output projection + residual -------------------
        es_x1 = ExitStack()
        p_x1 = es_x1.enter_context(tc.tile_pool(name="p_x1", side="left", bufs=1))
        x1 = p_x1.tile([P, TILES, D], f32, tag="x1")
        for t in range(TILES):
            ps = pA()
            for half, sl in ((0, slice(0, 512)), (1, slice(512, 768))):
                for c in range(DC):
                    nc.tensor.matmul(ps[:, sl],
                                     lhsT=aoutT[:, c, t * P:(t + 1) * P],
                                     rhs=aow_bf[:, c, sl],
                                     start=(c == 0), stop=(c == DC - 1))
            xr = xload.tile([P, D], f32, tag="xt")
            nc.sync.dma_start(xr, x_re[:, t, :])
            nc.vector.tensor_tensor(out=x1[:, t, :], in0=ps[:, :768], in1=xr,
                                    op=OP.add)
            nc.vector.tensor_tensor(out=x1[:, t, :], in0=x1[:, t, :], in1=aob,
                                    op=OP.add)
        es_aout.close()
        es_aow.close()

        es_dn2 = ExitStack()
        p_dn2 = es_dn2.enter_context(tc.tile_pool(name="p_dn2", side="left", bufs=1))
        downw_bf2 = p_dn2.tile([P, FC - FC1, D], bf16, tag="downw2")
        for c2 in range(FC1 // 2, FC // 2):
            st = stage.tile([P, 2, D], f32, tag="stage8k")
            nc.sync.dma_start(st, downw_re[:, c2 * 2:(c2 + 1) * 2, :])
            off = c2 * 2 - FC1
            nc.gpsimd.tensor_copy(out=downw_bf2[:, off:off + 2, :], in_=st)

        def downw_bf(m):
            return (downw_bf1[:, m, :] if m < FC1
                    else downw_bf2[:, m - FC1, :])

        # ---- Phase G: LN2 + transpose -------------------------------------
        es_n2 = ExitStack()
        p_n2 = es_n2.enter_context(tc.tile_pool(name="p_n2", side="right", bufs=1))
        n2T = p_n2.tile([P, DC, S], bf16, tag="n2T")
        for t in range(TILES):
            nt = work.tile([P, D], f32, tag="nt")
            layernorm_tile(x1[:, t, :], g2, b2, nt, aff=nc.gpsimd)
            for g in range(2):
                ps = pB()
                for j in range(3):
                    c = g * 3 + j
                    nc.tensor.transpose(ps[:, j * P:(j + 1) * P],
                                        nt[:, c * P:(c + 1) * P], id_f)
                pv = ps[:, :3 * P].rearrange("p (j q) -> p j q", j=3)
                nc.vector.tensor_copy(
                    out=n2T[:, g * 3:(g + 1) * 3, t * P:(t + 1) * P], in_=pv)

        # ---- Phase H: FFN --------------------------------------------------
        y_re = y_d.ap().rearrange("(t p) d -> p t d", p=P)
        QTR = 256
        for q4 in range(S // QTR):          # 4 quarters of 256 tokens
            pd = [pA() for _ in range(2)]   # two 128-token down psums
            for m in range(FC):
                psu = pB()
                for c in range(DC):
                    nc.tensor.matmul(
                        psu[:, :QTR],
                        lhsT=upw_bf[:, c, m * P:(m + 1) * P],
                        rhs=n2T[:, c, q4 * QTR:(q4 + 1) * QTR],
                        start=(c == 0), stop=(c == DC - 1))
                hs = work.tile([P, QTR], bf16, tag="hstrip")
                nc.scalar.activation(out=hs, in_=psu[:, :QTR], func=AF.Gelu,
                                     bias=upb[:, m:m + 1])
                for th in range(2):
                    for half, sl in ((0, slice(0, 512)), (1, slice(512, 768))):
                        nc.tensor.matmul(
                            pd[th][:, sl],
                            lhsT=hs[:, th * P:(th + 1) * P],
                            rhs=downw_bf(m)[:, sl],
                            start=(m == 0), stop=(m == FC - 1))
            for th in range(2):
                t = q4 * 2 + th
                ot = xload.tile([P, D], f32, tag="xt")
                nc.vector.tensor_tensor(out=ot, in0=pd[th][:, :768],
                                        in1=x1[:, t, :], op=OP.add)
                nc.vector.tensor_tensor(out=ot, in0=ot, in1=downb,
                                        op=OP.add)
                nc.sync.dma_start(y_re[:, t, :], ot)

        es_dn2.close()
        es_x1.close()
        es_dn1.close()
        es_n2.close()
        es_up.close()

    nc.compile()
    return nc


def _get_nc():
    if "nc" not in _CACHE:
        _CACHE["nc"] = _build()
    return _CACHE["nc"]


def _make_runner():
    """Cached PJRT executor for the SPMD bass kernel (8 cores).

    Modeled on concourse.bass2jax.run_bass_via_pjrt's multi-core path, but
    keeps the jitted function so repeat calls don't re-trace, and exposes a
    timing hook.
    """
    import jax
    import concourse.mybir as mybir
    from concourse import bass2jax
    from jax.experimental.shard_map import shard_map
    from jax.sharding import Mesh, PartitionSpec

    nc = _get_nc()
    bass2jax.install_neuronx_cc_hook()

    partition_name = (nc.partition_id_tensor.name
                      if nc.partition_id_tensor else None)
    in_names, out_names, out_avals, zero_outs = [], [], [], []
    for alloc in nc.m.functions[0].allocations:
        if not isinstance(alloc, mybir.MemoryLocationSet):
            continue
        name = alloc.memorylocations[0].name
        if alloc.kind == "ExternalInput":
            if name != partition_name:
                in_names.append(name)
        elif alloc.kind == "ExternalOutput":
            shape = tuple(alloc.tensor_shape)
            dtype = mybir.dt.np(alloc.dtype)
            out_names.append(name)
            out_avals.append(jax.core.ShapedArray(shape, dtype))
            zero_outs.append(np.zeros((NCORES * shape[0], *shape[1:]), dtype))
    n_params = len(in_names)
    n_outs = len(out_avals)
    all_in_names = list(in_names) + list(out_names)
    if partition_name is not None:
        all_in_names.append(partition_name)
    donate = tuple(range(n_params, n_params + n_outs))

    def _body(*args):
        operands = list(args)
        if partition_name is not None:
            operands.append(bass2jax.partition_id_tensor())
        outs = bass2jax._bass_exec_p.bind(
            *operands,
            out_avals=tuple(out_avals),
            in_names=tuple(all_in_names),
            out_names=tuple(out_names),
            lowering_input_output_aliases=(),
            sim_require_finite=True,
            sim_require_nnan=True,
            nc=nc,
        )
        return tuple(outs)

    devices = jax.devices()[:NCORES]
    mesh = Mesh(np.asarray(devices), ("core",))
    in_specs = (PartitionSpec("core"),) * (n_params + n_outs)
    out_specs = (PartitionSpec("core"),) * n_outs
    sharded = jax.jit(
        shard_map(_body, mesh=mesh, in_specs=in_specs, out_specs=out_specs,
                  check_rep=False),
        donate_argnums=donate, keep_unused=True)

    def run(in_maps, timing_iters=0):
        concat_in = [
            np.concatenate([np.asarray(in_maps[c][n]) for c in range(NCORES)],
                           axis=0)
            for n in in_names
        ]
        zeros = [z.copy() for z in zero_outs]
        _CACHE["concat_in"] = concat_in
        if "compiled" not in _CACHE:
            # AOT-compile so the NEFF can be dumped for profiling.
            _CACHE["compiled"] = sharded.lower(*concat_in, *zeros).compile()
            _CACHE["mesh"] = mesh
            _CACHE["zero_outs"] = zero_outs
        fn = _CACHE["compiled"]
        out = fn(*concat_in, *zeros)
        jax.block_until_ready(out)
        results = [np.asarray(o) for o in out]
        if timing_iters:
            import time
            from jax.sharding import NamedSharding
            dev_in = [jax.device_put(a, NamedSharding(mesh, PartitionSpec("core")))
                      for a in concat_in]
            times = []
            for _ in range(timing_iters):
                zs = [jax.device_put(z, NamedSharding(mesh, PartitionSpec("core")))
                      for z in zero_outs]
                jax.block_until_ready(zs)
                t0 = time.perf_counter()
                o = fn(*dev_in, *zs)
                jax.block_until_ready(o)
                times.append(time.perf_counter() - t0)
            _CACHE["times"] = times
        return {name: results[i] for i, name in enumerate(out_names)}

    return run


def _get_runner():
    if "runner" not in _CACHE:
        _CACHE["runner"] = _make_runner()
    return _CACHE["runner"]


def kernel(**inputs) -> np.ndarray:
    run = _get_runner()
    x = np.ascontiguousarray(np.asarray(inputs["x"], dtype=np.float32))
    weights = {k: np.ascontiguousarray(np.asarray(v, dtype=np.float32))
               for k, v in inputs.items() if k != "x"}
    in_maps = [dict(weights, x=np.ascontiguousarray(x[b])) for b in range(B)]
    out = run(in_maps, timing_iters=int(os.environ.get("KTIME", "0")))
    return out["y"].reshape(NCORES, S, D)



# revision 34
# speedup vs baseline: 1.0447x; 1.0447x over previous
"""Trainium2 Bass kernel for nn_DAWNLayer (moe_routing).

Strategy: data-parallel over batch B=8 across the 8 NeuronCores; each core
runs the full layer for one [S=1024, D=768] slice. Weights are replicated.

Key algorithmic facts used (verified numerically against the reference):
  * attn.mean(-1) of a softmax over the same axis is the constant 1/S, so
    `context` is a per-neuron constant: sigmoid(ctx_pat.sum(-1)/S). It is
    folded into the neuron-embedding matrix (scales recipe_norm rows).
  * top-k(8) + softmax + gather reduces to a thresholded masked softmax over
    the 96 neuron scores followed by a dense [S,96] @ [96,32] matmul
    (vector.max returns the top-8 values per row in one DVE instruction).
  * softmax without max-subtraction is safe here (|scores/8| < ~6).

Numerics: bf16 matmuls (fp32 PSUM accumulation) everywhere except the routing
score path (semantic scores), which stays exact fp32 so top-8 selection
matches the reference.

Attention is computed K-major: scores^T = K_tile^T-major matmul so exp writes
the AV-ready [k, q] layout straight from PSUM (no PE transposes / SBUF
copies), V carries a ones-column per head so the AV matmul also emits the
softmax denominator, and normalization is deferred to one small multiply per
(head, q-half) after AV. Weight f32->bf16 casts are split across the scalar
and pool engines and issued early so they overlap compute instead of gating
the Q/K projections; FFN weights prefetch during attention.
"""

import os
import numpy as np

B, S, D = 8, 1024, 768
H, DH = 12, 64
DFF = 3072
NN, NB, RK, TOPK = 96, 32, 64, 8
P = 128
TILES = S // P          # 8 token tiles
DC = D // P             # 6 chunks of d-model
FC = DFF // P           # 24 chunks of d_ff
NCORES = 8

_CACHE = {}


def _build():
    import concourse.bass as bass
    import concourse.bacc as bacc
    import concourse.mybir as mybir
    from concourse.tile import TileContext
    from concourse.masks import make_identity
    from contextlib import ExitStack

    f32 = mybir.dt.float32
    bf16 = mybir.dt.bfloat16
    AF = mybir.ActivationFunctionType
    OP = mybir.AluOpType
    AX = mybir.AxisListType

    nc = bacc.Bacc("TRN2", target_bir_lowering=False, debug=False,
                   num_devices=NCORES)

    # ---- DRAM I/O ----------------------------------------------------------
    d_in = {}
    def din(name, shape):
        d_in[name] = nc.dram_tensor(name, list(shape), f32, kind="ExternalInput")
        return d_in[name]

    x_d = din("x", (S, D))
    qw_d = din("q_w", (D, D)); qb_d = din("q_b", (D,))
    kw_d = din("k_w", (D, D)); kb_d = din("k_b", (D,))
    aow_d = din("ao_w", (D, D)); aob_d = din("ao_b", (D,))
    recipe_d = din("recipe", (NN, NB))
    ctx_d = din("ctx_pat", (NN, H))
    bemb_d = din("basis_emb", (NB, D))
    bA_d = din("basis_A", (NB, D, RK))
    voutw_d = din("vout_w", (RK, D)); voutb_d = din("vout_b", (D,))
    upw_d = din("up_w", (D, DFF)); upb_d = din("up_b", (DFF,))
    downw_d = din("down_w", (DFF, D)); downb_d = din("down_b", (D,))
    n1g_d = din("n1_g", (D,)); n1b_d = din("n1_b", (D,))
    n2g_d = din("n2_g", (D,)); n2b_d = din("n2_b", (D,))
    y_d = nc.dram_tensor("y", [S, D], f32, kind="ExternalOutput")

    def bcast_row(src_handle):
        """DRAM [D] -> AP broadcasting along 128 partitions."""
        ap = src_handle.ap()
        return bass.AP(tensor=ap.tensor, offset=ap.offset,
                       ap=[[0, P]] + list(ap.ap))

    with TileContext(nc, pool_alloc_mode="queue") as tc, ExitStack() as top:
        psA = top.enter_context(tc.tile_pool(name="psA", bufs=2, space="PSUM"))
        psB = top.enter_context(tc.tile_pool(name="psB", bufs=3, space="PSUM"))
        psT = top.enter_context(tc.tile_pool(name="psT", bufs=1, space="PSUM"))
        singles = top.enter_context(tc.tile_pool(name="singles", side="left", bufs=1))
        work = top.enter_context(tc.tile_pool(name="work", side="left", bufs=2))
        stage = top.enter_context(tc.tile_pool(name="stage", side="left", bufs=2))
        xload = top.enter_context(tc.tile_pool(name="xload", side="left", bufs=1))

        def pA():   return psA.tile([P, 1024], f32, tag="psA", name="psA_t")
        def pB():   return psB.tile([P, 512], f32, tag="psB", name="psB_t")
        def pT(dt): return psT.tile([P, P], dt, tag="psT", name="psT_t")

        # ---- constants / small tables -------------------------------------
        id_f = singles.tile([P, P], f32)
        make_identity(nc, id_f)
        eps_t = singles.tile([P, 1], f32)
        nc.vector.memset(eps_t, 1e-5)

        ones_row = singles.tile([1, P], f32)
        nc.vector.memset(ones_row, 1.0)

        def bcast_via_pe(dram, name, dt=f32):
            row = stage.tile([1, D], f32, tag="stage8k", name=name + "_row")
            nc.sync.dma_start(row, dram.ap()[None, :])
            out = singles.tile([P, D], dt, name=name)
            ps = pA()
            nc.tensor.matmul(ps[:, 0:512], lhsT=ones_row, rhs=row[:, 0:512],
                             start=True, stop=True)
            nc.tensor.matmul(ps[:, 512:768], lhsT=ones_row, rhs=row[:, 512:768],
                             start=True, stop=True)
            nc.vector.tensor_copy(out=out, in_=ps[:, :D])
            return out

        g1 = bcast_via_pe(n1g_d, "g1")
        b1 = bcast_via_pe(n1b_d, "b1")
        g2 = bcast_via_pe(n2g_d, "g2", bf16)
        b2 = bcast_via_pe(n2b_d, "b2", bf16)
        aob = bcast_via_pe(aob_d, "aob", bf16)
        voutb = bcast_via_pe(voutb_d, "voutb", bf16)
        downb = bcast_via_pe(downb_d, "downb", bf16)
        qb = singles.tile([P, DC], f32)
        nc.sync.dma_start(qb, qb_d.ap().rearrange("(o p) -> p o", p=P))
        kb = singles.tile([P, DC], f32)
        nc.sync.dma_start(kb, kb_d.ap().rearrange("(o p) -> p o", p=P))
        upb = singles.tile([P, FC], f32)
        nc.sync.dma_start(upb, upb_d.ap().rearrange("(o p) -> p o", p=P))

        # routing tables
        recipe_sb = singles.tile([NN, NB], f32)
        nc.sync.dma_start(recipe_sb, recipe_d.ap())
        ctx_sb = singles.tile([NN, H], f32)
        nc.sync.dma_start(ctx_sb, ctx_d.ap())
        bemb_sb = stage.tile([NB, D], f32, tag="stage8k")
        nc.sync.dma_start(bemb_sb, bemb_d.ap())
        voutw_sb = stage.tile([RK, D], f32, tag="stage8k")
        nc.sync.dma_start(voutw_sb, voutw_d.ap())
        voutw_bf = singles.tile([RK, D], bf16)
        nc.gpsimd.tensor_copy(out=voutw_bf, in_=voutw_sb)

        # recipe softmax (rows)
        rmax = singles.tile([NN, 1], f32)
        nc.vector.tensor_reduce(out=rmax, in_=recipe_sb, axis=AX.X, op=OP.max)
        nrmax = singles.tile([NN, 1], f32)
        nc.vector.tensor_scalar_mul(nrmax, rmax, -1.0)
        recipe_norm = singles.tile([NN, NB], f32)
        rsum = singles.tile([NN, 1], f32)
        nc.scalar.activation(out=recipe_norm, in_=recipe_sb, func=AF.Exp,
                             bias=nrmax[:, 0:1], accum_out=rsum)
        rinv = singles.tile([NN, 1], f32)
        nc.vector.reciprocal(rinv, rsum)
        nc.vector.tensor_scalar_mul(recipe_norm, recipe_norm, rinv[:, 0:1])

        # constant context -> sigmoid, folded into scaled recipe rows
        csum = singles.tile([NN, 1], f32)
        nc.vector.tensor_reduce(out=csum, in_=ctx_sb, axis=AX.X, op=OP.add)
        sigc = singles.tile([NN, 1], f32)
        nc.scalar.activation(out=sigc, in_=csum, func=AF.Sigmoid, scale=1.0 / S)
        recipe_s = singles.tile([NN, NB], f32)
        nc.vector.tensor_scalar_mul(recipe_s, recipe_norm, sigc[:, 0:1])

        # recipe_s^T [NB, NN]
        ps = pT(f32)
        nc.tensor.transpose(ps[:NB, :NN], recipe_s, id_f[:NN, :NN])
        recipe_sT = singles.tile([NB, NN], f32)
        nc.vector.tensor_copy(out=recipe_sT, in_=ps[:NB, :NN])

        # neuron_embT_scaled [D(part), NN] = basis_emb^T @ recipe_s^T
        es_rt = ExitStack()
        p_rt = es_rt.enter_context(tc.tile_pool(name="p_rt", side="left", bufs=1))
        nembT = p_rt.tile([P, DC, NN], f32, tag="nembT")
        for c in range(DC):
            ps = pB()
            nc.tensor.matmul(ps[:P, :NN], lhsT=bemb_sb[:NB, c * P:(c + 1) * P],
                             rhs=recipe_sT[:NB, :NN], start=True, stop=True)
            nc.vector.tensor_copy(out=nembT[:, c, :], in_=ps[:P, :NN])

        # ---- Phase A: LN1 + transpose -------------------------------------
        es_lnf = ExitStack()
        p_lnf = es_lnf.enter_context(tc.tile_pool(name="p_lnf", side="left", bufs=1))
        es_lnb = ExitStack()
        p_lnb = es_lnb.enter_context(tc.tile_pool(name="p_lnb", side="right", bufs=1))
        normT = p_lnf.tile([P, DC, S], f32, tag="normT")
        normTb = p_lnb.tile([P, DC, S], bf16, tag="normTb")

        def layernorm_tile(xt, g, b, out_tile, aff=None):
            """token-major LN: xt [128, 768] -> out_tile [128, 768] f32."""
            if aff is None:
                aff = nc.vector
            stats = work.tile([P, 3, 6], f32, tag="ln_stats")
            xv = xt.rearrange("p (a q) -> p a q", a=3)
            for a in range(3):
                nc.vector.bn_stats(out=stats[:, a, :], in_=xv[:, a, :])
            mv = work.tile([P, 2], f32, tag="ln_mv")
            nc.vector.bn_aggr(out=mv, in_=stats)
            std = work.tile([P, 1], f32, tag="ln_std")
            nc.scalar.activation(out=std, in_=mv[:, 1:2], func=AF.Sqrt,
                                 bias=eps_t[:, 0:1])
            rstd = work.tile([P, 1], f32, tag="ln_rstd")
            nc.vector.reciprocal(rstd, std)
            nc.vector.tensor_scalar(out=out_tile, in0=xt, scalar1=mv[:, 0:1],
                                    scalar2=rstd[:, 0:1], op0=OP.subtract,
                                    op1=OP.mult)
            aff.tensor_tensor(out=out_tile, in0=out_tile, in1=g, op=OP.mult)
            aff.tensor_tensor(out=out_tile, in0=out_tile, in1=b, op=OP.add)

        x_re = x_d.ap().rearrange("(t p) d -> p t d", p=P)
        for t in range(TILES):
            xt = xload.tile([P, D], f32, tag="xt")
            nc.sync.dma_start(xt, x_re[:, t, :])
            nt = work.tile([P, D], f32, tag="nt")
            layernorm_tile(xt, g1, b1, nt)
            for g in range(2):
                ps = pB()
                for j in range(3):
                    c = g * 3 + j
                    nc.tensor.transpose(ps[:, j * P:(j + 1) * P],
                                        nt[:, c * P:(c + 1) * P], id_f)
                pv = ps[:, :3 * P].rearrange("p (j q) -> p j q", j=3)
                nc.vector.tensor_copy(
                    out=normT[:, g * 3:(g + 1) * 3, t * P:(t + 1) * P], in_=pv)
        for c in range(DC):
            if c % 2 == 0:
                nc.scalar.activation(out=normTb[:, c, :], in_=normT[:, c, :],
                                     func=AF.Identity)
            else:
                nc.gpsimd.tensor_copy(out=normTb[:, c, :], in_=normT[:, c, :])

        # q/k weight load + bf16 cast, issued early so the casts fill the
        # scalar/pool idle windows during phases B/C and never gate Phase D.
        es_qkw = ExitStack()
        p_qkw = es_qkw.enter_context(tc.tile_pool(name="p_qkw", side="right", bufs=2))

        def load_w_bf(dram, shape_free, tag):
            """Load [D, N] fp32 weight as [128, DC, N] bf16 (split-engine cast)."""
            n = shape_free
            w_bf = p_qkw.tile([P, DC, n], bf16, tag=tag)
            re = dram.ap().rearrange("(o p) n -> p o n", p=P)
            for c in range(DC):
                st = stage.tile([P, 1, n], f32, tag="stage8k")
                nc.sync.dma_start(st[:, 0, :], re[:, c, :])
                if c % 2 == 0:
                    nc.scalar.activation(out=w_bf[:, c, :], in_=st[:, 0, :],
                                         func=AF.Identity)
                else:
                    nc.gpsimd.tensor_copy(out=w_bf[:, c, :], in_=st[:, 0, :])
            return w_bf

        qw_bf = load_w_bf(qw_d, D, "qkw")
        kw_bf = load_w_bf(kw_d, D, "qkw")

        # ---- Phase B: routing scores -> token_recipe ----------------------
        tr_sb = singles.tile([P, TILES, NB], f32)      # token-major token_recipe
        maskT = p_rt.tile([NN, S], f32, tag="maskT")   # mask_w^T
        for t in range(TILES):
            ps = pB()
            for c in range(DC):
                nc.tensor.matmul(ps[:P, :NN], lhsT=normT[:, c, t * P:(t + 1) * P],
                                 rhs=nembT[:, c, :], start=(c == 0),
                                 stop=(c == DC - 1))
            fin = work.tile([P, NN], f32, tag="fin")
            nc.vector.tensor_copy(out=fin, in_=ps[:P, :NN])
            mx = work.tile([P, 8], f32, tag="mx")
            nc.vector.max(out=mx, in_=fin)
            nmx = work.tile([P, 1], f32, tag="nmx")
            nc.vector.tensor_scalar_mul(nmx, mx[:, 0:1], -1.0)
            e = work.tile([P, NN], f32, tag="e")
            nc.scalar.activation(out=e, in_=fin, func=AF.Exp, bias=nmx[:, 0:1])
            msk = work.tile([P, NN], f32, tag="msk")
            nc.vector.tensor_scalar(out=msk, in0=fin, scalar1=mx[:, 7:8],
                                    scalar2=None, op0=OP.is_ge)
            nc.vector.tensor_tensor(out=e, in0=e, in1=msk, op=OP.mult)
            den = work.tile([P, 1], f32, tag="den")
            nc.vector.tensor_reduce(out=den, in_=e, axis=AX.X, op=OP.add)
            idn = work.tile([P, 1], f32, tag="idn")
            nc.vector.reciprocal(idn, den)
            nc.vector.tensor_scalar_mul(e, e, idn[:, 0:1])
            # transpose mask_w -> maskT
            ps2 = pT(f32)
            nc.tensor.transpose(ps2[:NN, :P], e[:, :NN], id_f)
            nc.vector.tensor_copy(out=maskT[:, t * P:(t + 1) * P], in_=ps2[:NN, :P])
        for t in range(TILES):
            ps = pB()
            nc.tensor.matmul(ps[:P, :NB], lhsT=maskT[:, t * P:(t + 1) * P],
                             rhs=recipe_norm, start=True, stop=True)
            nc.vector.tensor_copy(out=tr_sb[:, t, :], in_=ps[:P, :NB])
        es_lnf.close()
        es_rt.close()

        # ---- Phase C: proj / v_sem / V ------------------------------------
        es_bas = ExitStack()
        p_bas = es_bas.enter_context(tc.tile_pool(name="p_bas", side="left", bufs=1))
        basis_bf = p_bas.tile([P, DC, NB, RK], bf16, tag="basis")
        bA_re = bA_d.ap().rearrange("n (o p) r -> p o n r", p=P)
        for c in range(DC):
            for nh in range(2):
                st = stage.tile([P, NB // 2, RK], f32, tag="stage8k")
                nsl = slice(nh * (NB // 2), (nh + 1) * (NB // 2))
                nc.sync.dma_start(st, bA_re[:, c, nsl, :])
                if (c * 2 + nh) % 2 == 0:
                    nc.scalar.activation(out=basis_bf[:, c, nsl, :], in_=st,
                                         func=AF.Identity)
                else:
                    nc.gpsimd.tensor_copy(out=basis_bf[:, c, nsl, :], in_=st)

        vsemT_bf = singles.tile([RK, TILES, P], bf16)

        NGRP = 4           # groups of 8 basis entries -> 512-wide psum
        GN = NB // NGRP    # 8
        for t in range(TILES):
            vpart = work.tile([P, NGRP, RK], bf16, tag="vpart")
            for g in range(NGRP):
                ps = pB()
                for c in range(DC):
                    nc.tensor.matmul(
                        ps[:, :512],
                        lhsT=normTb[:, c, t * P:(t + 1) * P],
                        rhs=basis_bf[:, c, g * GN:(g + 1) * GN, :],
                        start=(c == 0), stop=(c == DC - 1))
                sc = work.tile([P, GN, RK], bf16, tag="sc")
                nc.vector.tensor_tensor(
                    out=sc, in0=ps.rearrange("p (n r) -> p n r", n=GN),
                    in1=tr_sb[:, t, g * GN:(g + 1) * GN, None].to_broadcast(
                        [P, GN, RK]),
                    op=OP.mult)
                nc.vector.tensor_tensor(out=sc[:, 0:4, :], in0=sc[:, 0:4, :],
                                        in1=sc[:, 4:8, :], op=OP.add)
                nc.vector.tensor_tensor(out=sc[:, 0:2, :], in0=sc[:, 0:2, :],
                                        in1=sc[:, 2:4, :], op=OP.add)
                nc.vector.tensor_tensor(out=vpart[:, g, :], in0=sc[:, 0, :],
                                        in1=sc[:, 1, :], op=OP.add)
            vsem = work.tile([P, RK], f32, tag="vsem")
            nc.vector.tensor_tensor(out=vsem, in0=vpart[:, 0, :],
                                    in1=vpart[:, 1, :], op=OP.add)
            nc.vector.tensor_tensor(out=vsem, in0=vsem, in1=vpart[:, 2, :],
                                    op=OP.add)
            nc.vector.tensor_tensor(out=vsem, in0=vsem, in1=vpart[:, 3, :],
                                    op=OP.add)
            ps2 = pT(f32)
            nc.tensor.transpose(ps2[:RK, :P], vsem, id_f)
            nc.vector.tensor_copy(out=vsemT_bf[:, t, :], in_=ps2[:RK, :P])
        es_bas.close()

        es_dn1 = ExitStack()
        p_dn1 = es_dn1.enter_context(tc.tile_pool(name="p_dn1", side="left", bufs=1))
        FC1 = 10
        downw_bf1 = p_dn1.tile([P, FC1, D], bf16, tag="downw1")
        downw_re = downw_d.ap().rearrange("(o p) n -> p o n", p=P)
        for c2 in range(FC1 // 2):
            st = stage.tile([P, 2, D], f32, tag="stage8k")
            nc.sync.dma_start(st, downw_re[:, c2 * 2:(c2 + 1) * 2, :])
            nc.gpsimd.tensor_copy(out=downw_bf1[:, c2 * 2:c2 * 2 + 2, :], in_=st)

        es_at = ExitStack()
        p_at = es_at.enter_context(tc.tile_pool(name="p_at", side="left", bufs=2))

        # V with a ones-column per head (col h*65+64) so the AV matmul also
        # produces the softmax denominator (row DH of each head's AV psum).
        es_v = ExitStack()
        p_v = es_v.enter_context(tc.tile_pool(name="p_v", side="left", bufs=1))
        EH = DH + 1
        V_ext = p_v.tile([P, TILES, H * EH], bf16, tag="V")
        ones_view = V_ext.rearrange("p t (h e) -> p t h e", e=EH)[:, :, :, DH:EH]
        nc.gpsimd.memset(ones_view, 1.0)
        for t in range(TILES):
            ps = pA()
            nc.tensor.matmul(ps[:, 0:512], lhsT=vsemT_bf[:, t, :],
                             rhs=voutw_bf[:, 0:512], start=True, stop=True)
            nc.tensor.matmul(ps[:, 512:768], lhsT=vsemT_bf[:, t, :],
                             rhs=voutw_bf[:, 512:768], start=True, stop=True)
            vv = V_ext[:, t, :].rearrange("p (h e) -> p h e", e=EH)
            nc.vector.tensor_tensor(
                out=vv[:, :, 0:DH],
                in0=ps[:, :768].rearrange("p (h e) -> p h e", e=DH),
                in1=voutb.rearrange("p (h e) -> p h e", e=DH),
                op=OP.add)

        # ---- Phase D: Q/K projections -------------------------------------
        es_qt = ExitStack()
        p_qt = es_qt.enter_context(tc.tile_pool(name="p_qt", side="left", bufs=2))

        QT = p_qt.tile([P, DC, S], bf16, tag="qkt")
        KT = p_qt.tile([P, DC, S], bf16, tag="qkt")
        for (w_bf, bias, out_t) in ((qw_bf, qb, QT), (kw_bf, kb, KT)):
            for m in range(DC):
                ps = pA()
                for half in range(2):
                    sl = slice(half * 512, (half + 1) * 512)
                    for c in range(DC):
                        nc.tensor.matmul(ps[:, sl],
                                         lhsT=w_bf[:, c, m * P:(m + 1) * P],
                                         rhs=normTb[:, c, sl],
                                         start=(c == 0), stop=(c == DC - 1))
                nc.vector.tensor_scalar(out=out_t[:, m, :], in0=ps,
                                        scalar1=bias[:, m:m + 1], scalar2=None,
                                        op0=OP.add)
        es_qkw.close()
        es_lnb.close()

        # prefetch FFN up weights during attention
        es_up = ExitStack()
        p_up = es_up.enter_context(tc.tile_pool(name="p_up", side="right", bufs=1))
        upw_bf = p_up.tile([P, DC, DFF], bf16, tag="upw")
        upw_re = upw_d.ap().rearrange("(o p) n -> p o n", p=P)
        for c in range(DC):
            for half in range(2):
                st = stage.tile([P, 1, DFF // 2], f32, tag="stage8k")
                sl = slice(half * (DFF // 2), (half + 1) * (DFF // 2))
                nc.sync.dma_start(st[:, 0, :], upw_re[:, c, sl])
                nc.gpsimd.tensor_copy(out=upw_bf[:, c, sl], in_=st[:, 0, :])

        # prefetch attn-output projection weights too
        es_aow = ExitStack()
        p_aow = es_aow.enter_context(tc.tile_pool(name="p_aow", side="right", bufs=1))
        aow_bf = p_aow.tile([P, DC, D], bf16, tag="aow")
        aow_re = aow_d.ap().rearrange("(o p) n -> p o n", p=P)
        for c in range(DC):
            st = stage.tile([P, 1, D], f32, tag="stage8k")
            nc.sync.dma_start(st[:, 0, :], aow_re[:, c, :])
            nc.gpsimd.tensor_copy(out=aow_bf[:, c, :], in_=st[:, 0, :])

        # ---- Phase E: attention (transposed scores, deferred normalize) ----
        # Scores are computed K-major ([k, q]) so exp writes the AV-ready
        # layout directly from PSUM (no PE transposes, no SBUF copies). The
        # softmax denominator comes from V's ones-column (row DH of the AV
        # psum); normalization is one PSUM*PSUM multiply per (h, q-half).
        es_aout = ExitStack()
        p_aout = es_aout.enter_context(tc.tile_pool(name="p_aout", side="right", bufs=1))
        aoutT = p_aout.tile([P, DC, S], bf16, tag="aoutT")
        for h in range(H):
            hp = (h % 2) * DH
            hc = h // 2
            attnT = p_at.tile([P, TILES, S], bf16, tag="attnT")
            for kt in range(TILES):
                ps = pA()
                for half in range(2):
                    sl = slice(half * 512, (half + 1) * 512)
                    nc.tensor.matmul(ps[:, sl],
                                     lhsT=KT[hp:hp + DH, hc, kt * P:(kt + 1) * P],
                                     rhs=QT[hp:hp + DH, hc, sl],
                                     start=True, stop=True)
                nc.scalar.activation(out=attnT[:, kt, :], in_=ps, func=AF.Exp,
                                     scale=0.125)
            for qc in range(2):
                qsl = slice(qc * 512, (qc + 1) * 512)
                ps = pB()
                for tb in range(TILES):
                    nc.tensor.matmul(ps[:DH + 1, :],
                                     lhsT=V_ext[:, tb, h * EH:(h + 1) * EH],
                                     rhs=attnT[:, tb, qsl],
                                     start=(tb == 0), stop=(tb == TILES - 1))
                av = work.tile([DH + 1, 512], f32, tag="dvb")
                nc.vector.tensor_copy(out=av, in_=ps[:DH + 1, :512])
                dv = work.tile([1, 512], f32, tag="dv")
                nc.vector.reciprocal(dv, av[DH:DH + 1, :])
                dvb = work.tile([DH, 512], f32, tag="dvb")
                nc.gpsimd.partition_broadcast(dvb, dv)
                nc.vector.tensor_tensor(out=aoutT[hp:hp + DH, hc, qsl],
                                        in0=av[:DH, :], in1=dvb,
                                        op=OP.mult)
        es_qt.close()
        es_v.close()
        es_at.close()

        # ---- Phase F: attn output projection + residual -------------------
        es_x1 = ExitStack()
        p_x1 = es_x1.enter_context(tc.tile_pool(name="p_x1", side="left", bufs=1))
        x1 = p_x1.tile([P, TILES, D], f32, tag="x1")
        for t in range(TILES):
            ps = pA()
            for half, sl in ((0, slice(0, 512)), (1, slice(512, 768))):
                for c in range(DC):
                    nc.tensor.matmul(ps[:, sl],
                                     lhsT=aoutT[:, c, t * P:(t + 1) * P],
                                     rhs=aow_bf[:, c, sl],
                                     start=(c == 0), stop=(c == DC - 1))
            xr = xload.tile([P, D], f32, tag="xt")
            nc.sync.dma_start(xr, x_re[:, t, :])
            nc.vector.tensor_tensor(out=x1[:, t, :], in0=ps[:, :768], in1=xr,
                                    op=OP.add)
            nc.vector.tensor_tensor(out=x1[:, t, :], in0=x1[:, t, :], in1=aob,
                                    op=OP.add)
        es_aout.close()
        es_aow.close()

        es_dn2 = ExitStack()
        p_dn2 = es_dn2.enter_context(tc.tile_pool(name="p_dn2", side="left", bufs=1))
        downw_bf2 = p_dn2.tile([P, FC - FC1, D], bf16, tag="downw2")
        for c2 in range(FC1 // 2, FC // 2):
            st = stage.tile([P, 2, D], f32, tag="stage8k")
            nc.sync.dma_start(st, downw_re[:, c2 * 2:(c2 + 1) * 2, :])
            off = c2 * 2 - FC1
            nc.gpsimd.tensor_copy(out=downw_bf2[:, off:off + 2, :], in_=st)

        def downw_bf(m):
            return (downw_bf1[:, m, :] if m < FC1
                    else downw_bf2[:, m - FC1, :])

        # ---- Phase G: LN2 + transpose -------------------------------------
        es_n2 = ExitStack()
        p_n2 = es_n2.enter_context(tc.tile_pool(name="p_n2", side="right", bufs=1))
        n2T = p_n2.tile([P, DC, S], bf16, tag="n2T")
        for t in range(TILES):
            nt = work.tile([P, D], f32, tag="nt")
            layernorm_tile(x1[:, t, :], g2, b2, nt, aff=nc.gpsimd)
            for g in range(2):
                ps = pB()
                for j in range(3):
                    c = g * 3 + j
                    nc.tensor.transpose(ps[:, j * P:(j + 1) * P],
                                        nt[:, c * P:(c + 1) * P], id_f)
                pv = ps[:, :3 * P].rearrange("p (j q) -> p j q", j=3)
                nc.vector.tensor_copy(
                    out=n2T[:, g * 3:(g + 1) * 3, t * P:(t + 1) * P], in_=pv)

        # ---- Phase H: FFN --------------------------------------------------
        y_re = y_d.ap().rearrange("(t p) d -> p t d", p=P)
        QTR = 256
        for q4 in range(S // QTR):          # 4 quarters of 256 tokens
            pd = [pA() for _ in range(2)]   # two 128-token down psums
            for m in range(FC):
                psu = pB()
                for c in range(DC):
                    nc.tensor.matmul(
                        psu[:, :QTR],
                        lhsT=upw_bf[:, c, m * P:(m + 1) * P],
                        rhs=n2T[:, c, q4 * QTR:(q4 + 1) * QTR],
                        start=(c == 0), stop=(c == DC - 1))
                hs = work.tile([P, QTR], bf16, tag="hstrip")
                nc.scalar.activation(out=hs, in_=psu[:, :QTR], func=AF.Gelu,
                                     bias=upb[:, m:m + 1])
                for th in range(2):
                    for half, sl in ((0, slice(0, 512)), (1, slice(512, 768))):
                        nc.tensor.matmul(
                            pd[th][:, sl],
                            lhsT=hs[:, th * P:(th + 1) * P],
                            rhs=downw_bf(m)[:, sl],
                            start=(m == 0), stop=(m == FC - 1))
            for th in range(2):
                t = q4 * 2 + th
                ot = xload.tile([P, D], f32, tag="xt")
                nc.vector.tensor_tensor(out=ot, in0=pd[th][:, :768],
                                        in1=x1[:, t, :], op=OP.add)
                nc.vector.tensor_tensor(out=ot, in0=ot, in1=downb,
                                        op=OP.add)
                nc.sync.dma_start(y_re[:, t, :], ot)

        es_dn2.close()
        es_x1.close()
        es_dn1.close()
        es_n2.close()
        es_up.close()

    nc.compile()
    return nc


def _get_nc():
    if "nc" not in _CACHE:
        _CACHE["nc"] = _build()
    return _CACHE["nc"]


def _make_runner():
    """Cached PJRT executor for the SPMD bass kernel (8 cores).

    Modeled on concourse.bass2jax.run_bass_via_pjrt's multi-core path, but
    keeps the jitted function so repeat calls don't re-trace, and exposes a
    timing hook.
    """
    import jax
    import concourse.mybir as mybir
    from concourse import bass2jax
    from jax.experimental.shard_map import shard_map
    from jax.sharding import Mesh, PartitionSpec

    nc = _get_nc()
    bass2jax.install_neuronx_cc_hook()

    partition_name = (nc.partition_id_tensor.name
                      if nc.partition_id_tensor else None)
    in_names, out_names, out_avals, zero_outs = [], [], [], []
    for alloc in nc.m.functions[0].allocations:
        if not isinstance(alloc, mybir.MemoryLocationSet):
            continue
        name = alloc.memorylocations[0].name
        if alloc.kind == "ExternalInput":
            if name != partition_name:
                in_names.append(name)
        elif alloc.kind == "ExternalOutput":
            shape = tuple(alloc.tensor_shape)
            dtype = mybir.dt.np(alloc.dtype)
            out_names.append(name)
            out_avals.append(jax.core.ShapedArray(shape, dtype))
            zero_outs.append(np.zeros((NCORES * shape[0], *shape[1:]), dtype))
    n_params = len(in_names)
    n_outs = len(out_avals)
    all_in_names = list(in_names) + list(out_names)
    if partition_name is not None:
        all_in_names.append(partition_name)
    donate = tuple(range(n_params, n_params + n_outs))

    def _body(*args):
        operands = list(args)
        if partition_name is not None:
            operands.append(bass2jax.partition_id_tensor())
        outs = bass2jax._bass_exec_p.bind(
            *operands,
            out_avals=tuple(out_avals),
            in_names=tuple(all_in_names),
            out_names=tuple(out_names),
            lowering_input_output_aliases=(),
            sim_require_finite=True,
            sim_require_nnan=True,
            nc=nc,
        )
        return tuple(outs)

    devices = jax.devices()[:NCORES]
    mesh = Mesh(np.asarray(devices), ("core",))
    in_specs = (PartitionSpec("core"),) * (n_params + n_outs)
    out_specs = (PartitionSpec("core"),) * n_outs
    sharded = jax.jit(
        shard_map(_body, mesh=mesh, in_specs=in_specs, out_specs=out_specs,
                  check_rep=False),
        donate_argnums=donate, keep_unused=True)

    def run(in_maps, timing_iters=0):
        concat_in = [
            np.concatenate([np.asarray(in_maps[c][n]) for c in range(NCORES)],
                           axis=0)
            for n in in_names
        ]
        zeros = [z.copy() for z in zero_outs]
        _CACHE["concat_in"] = concat_in
        if "compiled" not in _CACHE:
            # AOT-compile so the NEFF can be dumped for profiling.
            _CACHE["compiled"] = sharded.lower(*concat_in, *zeros).compile()
            _CACHE["mesh"] = mesh
            _CACHE["zero_outs"] = zero_outs
        fn = _CACHE["compiled"]
        out = fn(*concat_in, *zeros)
        jax.block_until_ready(out)
        results = [np.asarray(o) for o in out]
        if timing_iters:
            import time
            from jax.sharding import NamedSharding
            dev_in = [jax.device_put(a, NamedSharding(mesh, PartitionSpec("core")))
                      for a in concat_in]
            times = []
            for _ in range(timing_iters):
                zs = [jax.device_put(z, NamedSharding(mesh, PartitionSpec("core")))
                      for z in zero_outs]
                jax.block_until_ready(zs)
                t0 = time.perf_counter()
                o = fn(*dev_in, *zs)
                jax.block_until_ready(o)
                times.append(time.perf_counter() - t0)
            _CACHE["times"] = times
        return {name: results[i] for i, name in enumerate(out_names)}

    return run


def _get_runner():
    if "runner" not in _CACHE:
        _CACHE["runner"] = _make_runner()
    return _CACHE["runner"]


def kernel(**inputs) -> np.ndarray:
    run = _get_runner()
    x = np.ascontiguousarray(np.asarray(inputs["x"], dtype=np.float32))
    weights = {k: np.ascontiguousarray(np.asarray(v, dtype=np.float32))
               for k, v in inputs.items() if k != "x"}
    in_maps = [dict(weights, x=np.ascontiguousarray(x[b])) for b in range(B)]
    out = run(in_maps, timing_iters=int(os.environ.get("KTIME", "0")))
    return out["y"].reshape(NCORES, S, D)

